# revision 8
# baseline (speedup 1.0000x reference)
"""Trainium2 Bass kernel for nn_ComplexMixture.

Per batch element b (R = input_real[b] [S,D], I = input_imag[b] [S,D], w [S]):
    out_r = (w*R)^T R + (w*I)^T I        (symmetric)
    out_i = (w*I)^T R - (w*R)^T I        (antisymmetric)

Fold sqrt(w) into both operands (A = sqrt(w)*R, B = sqrt(w)*I) and use the
Gauss 3-multiplication complex product with E = A + B:
    M1 = A^T B,  M2 = B^T A,  M3 = E^T E
    out_r = M3 - M1 - M2
    out_i = M2 - M1
so each output block pair costs 3 PSUM-accumulated matmuls per contraction
chunk instead of 4 (25% less PE time). The combines run concurrently with
the matmul stream, split across engines (only the vector engine may pair a
PSUM read with a second tensor operand; gpsimd may not touch PSUM at all):
    scalar: u = fp16(M1), v = fp16(M2)     (PSUM->SBUF copies)
    vector: t = M3 - u,  out_r = t - v     (one PSUM operand each)
    gpsimd: out_i = v - u,  oin = 0 - out_i  (SBUF only)

Sharding: data-parallel over batch, one batch element per NeuronCore (B == 8
== n_cores). Each core runs the identical program on its own slice.

Host marshalling: R/I are cast to fp16 (halves input DMA bytes) and sqrt(w)
is precomputed on host (4K scalars). Matmuls run in fp16 with fp32 PSUM
accumulation; outputs are stored as fp16 (halves store DMA bytes) and upcast
on host. Measured L2 relative error vs the fp32 reference ~5e-4.

out_r is symmetric and out_i antisymmetric, so each strictly-lower [384,384]
block is skipped on device (the pass list covers only the upper block
triangle). The host unshard mirrors them with pure transpose copies: out_r's
directly, out_i's from the device-negated oin_out.

The PE HAM clock-gate releases only after ~6us of *continuous* matmul
activity; any gap restarts the wait and leaves the PE at half rate. So a
prewarm burst of dummy matmuls on zeroed tiles bridges the preamble barrier
to the first real matmuls, the first two passes run k-interleaved so late
input chunks can't open a gap, and short dummy fillers pad the k-chunk
seams.
"""

import sys
import types

import numpy as np

# If the environment requests tracing (BASS_TRACE=1) but the image lacks
# antenv.axon_hooks, bass_utils would crash importing it; provide a no-op
# hook registry so tracing degrades gracefully instead.
try:
    import antenv.axon_hooks  # noqa: F401
except ImportError:
    _hooks = types.ModuleType("antenv.axon_hooks")
    _hooks._hook = None
    _hooks.set_axon_ntff_profile_hook = lambda h: setattr(_hooks, "_hook", h)
    _hooks.get_axon_ntff_profile_hook = lambda: _hooks._hook
    sys.modules["antenv.axon_hooks"] = _hooks

import concourse.bacc as bacc
import concourse.bass_utils as bass_utils
import concourse.mybir as mybir
import concourse.tile as tile

B, S, D = 8, 512, 768
P = 128          # SBUF/PSUM partitions; matmul contraction tile
KC = S // P      # 4 contraction chunks per operand
MT = D // P      # 6 output row tiles
NW = 384         # matmul moving free dim (<=512 fp32 PSUM bank)
NB = D // NW     # 2 output column blocks
N_CORES = 8
N_PREWARM = 9    # dummy N=384 matmuls bridging the preamble barrier to the
                 # first real matmuls (~0.32us each at half rate)

# upper-block-triangle passes (m row tile, n 384-col block); strictly-lower
# blocks are mirrored on host from symmetry
PASSES = [(0, 0), (0, 1), (1, 0), (1, 1), (2, 0), (2, 1), (3, 1), (4, 1), (5, 1)]

_CACHE: dict = {}


def _build():
    f32, f16 = mybir.dt.float32, mybir.dt.float16
    BYP = mybir.AluOpType.bypass
    SUB = mybir.AluOpType.subtract
    nc = bacc.Bacc(
        "TRN2", target_bir_lowering=False, debug=False, num_devices=N_CORES
    )
    # Host-packed partition-major: r_in[p, k*D:(k+1)*D] = R[k*P+p, :], so a
    # whole k-chunk group is one DMA with long per-partition descriptors.
    r_d = nc.dram_tensor("r_in", [P, KC * D], f16, kind="ExternalInput").ap()
    i_d = nc.dram_tensor("i_in", [P, KC * D], f16, kind="ExternalInput").ap()
    # sqrt(w) chunks, partition-major (col k = chunk k's 128 scalars)
    s_d = nc.dram_tensor("s_in", [P, KC], f32, kind="ExternalInput").ap()
    or_d = nc.dram_tensor("or_out", [D, D], f16, kind="ExternalOutput").ap()
    oi_d = nc.dram_tensor("oi_out", [D, D], f16, kind="ExternalOutput").ap()
    # negated upper-right block of out_i; host transposes it into the
    # skipped lower-left block (out_i is antisymmetric)
    oin_d = nc.dram_tensor("oin_out", [D // 2, NW], f16, kind="ExternalOutput").ap()

    with tile.TileContext(nc) as tc:
        with (
            tc.tile_pool(name="const", bufs=1) as cpool,
            tc.tile_pool(name="stage", bufs=1) as spool,
            tc.tile_pool(name="abc", bufs=1) as apool,
            tc.tile_pool(name="tsb", bufs=2) as tpool,
            tc.tile_pool(name="osb", bufs=2) as opool,
            tc.tile_pool(name="ps", bufs=2, space="PSUM") as pspool,
            tc.tile_pool(name="pw", bufs=1, space="PSUM") as pwpool,
        ):
            # Scale vector first on the otherwise-idle gpsimd ring (tiny,
            # lands before the r3/i3 chunks queued behind it).
            s_t = cpool.tile([P, KC], f32, name="s_t")
            nc.gpsimd.dma_start(s_t[:], s_d)

            # PE prewarm (see module docstring).
            zw = cpool.tile([P, 4 * P], f16, name="zw")
            nc.vector.memset(zw[:], 0.0)
            pw_ps = pwpool.tile([P, 3 * P], f32, name="pw_ps", tag="pw")
            for _ in range(N_PREWARM):
                nc.tensor.matmul(
                    pw_ps[:], zw[:, 0:P], zw[:, P : 4 * P], start=True, stop=True
                )

            def filler(n_cols):
                nc.tensor.matmul(
                    pw_ps[:, 0:n_cols], zw[:, 0:P], zw[:, P : P + n_cols],
                    start=True, stop=True,
                )

            # One k-chunk per DMA, spread over all three rings in
            # consumption order, so no single slow ring can starve the
            # matmul head: sync gets k0/k1, scalar k2, gpsimd k3.
            rt, it = [], []
            for k in range(KC):
                rt.append(spool.tile([P, D], f16, name=f"r{k}", tag=f"r{k}"))
                it.append(spool.tile([P, D], f16, name=f"i{k}", tag=f"i{k}"))

            def dsl(k):
                return slice(k * D, (k + 1) * D)

            nc.sync.dma_start(rt[0][:], r_d[:, dsl(0)])
            nc.scalar.dma_start(rt[2][:], r_d[:, dsl(2)])
            nc.sync.dma_start(it[0][:], i_d[:, dsl(0)])
            nc.scalar.dma_start(it[2][:], i_d[:, dsl(2)])
            nc.sync.dma_start(rt[1][:], r_d[:, dsl(1)])
            nc.gpsimd.dma_start(rt[3][:], r_d[:, dsl(3)])
            nc.sync.dma_start(it[1][:], i_d[:, dsl(1)])
            nc.gpsimd.dma_start(it[3][:], i_d[:, dsl(3)])

            # Per-row scaling and E sums, all on VectorE in consumption
            # order (each ~0.42us; the DMA chunks arrive slower than the
            # queue drains, so each op starts right when its chunk lands).
            At, Bt, Et = [], [], []
            for k in range(KC):
                a = apool.tile([P, D], f16, name=f"A{k}", tag=f"A{k}")
                nc.vector.tensor_scalar_mul(a[:], rt[k][:], s_t[:, k : k + 1])
                b = apool.tile([P, D], f16, name=f"B{k}", tag=f"B{k}")
                nc.vector.tensor_scalar_mul(b[:], it[k][:], s_t[:, k : k + 1])
                e = apool.tile([P, D], f16, name=f"E{k}", tag=f"E{k}")
                nc.vector.tensor_add(e[:], a[:], b[:])
                At.append(a)
                Bt.append(b)
                Et.append(e)

            def nsl(n):
                return slice(n * NW, (n + 1) * NW)

            ps_of = {}

            def alloc(p):
                ps_of[p] = tuple(
                    pspool.tile([P, NW], f32, name=f"M{t_}_{p}", tag=f"M{t_}")
                    for t_ in (1, 2, 3)
                )

            def mm_group(p, k):
                """Emit the 3 matmuls of pass p for contraction chunk k."""
                m, n = PASSES[p]
                ms = slice(m * P, (m + 1) * P)
                M1, M2, M3 = ps_of[p]
                st, sp = (k == 0), (k == KC - 1)
                # stop group ordered M3,M1,M2 so the evac chain (u needs M1,
                # t needs M3) can start before M2's last matmul retires
                order = (
                    [(M3, Et[k], Et[k]), (M1, At[k], Bt[k]), (M2, Bt[k], At[k])]
                    if sp
                    else [(M1, At[k], Bt[k]), (M2, Bt[k], At[k]), (M3, Et[k], Et[k])]
                )
                for dst, lt, rh in order:
                    nc.tensor.matmul(
                        dst[:], lt[:, ms], rh[:, nsl(n)], start=st, stop=sp
                    )

            def evac(p):
                """Combine pass p's PSUM banks and store (fp16)."""
                m, n = PASSES[p]
                ms = slice(m * P, (m + 1) * P)
                M1, M2, M3 = ps_of[p]
                u = tpool.tile([P, NW], f16, name=f"u{p}", tag="u")
                v = tpool.tile([P, NW], f16, name=f"v{p}", tag="v")
                t = tpool.tile([P, NW], f32, name=f"t{p}", tag="t")
                or_sb = opool.tile([P, NW], f16, name=f"or{p}", tag="or_sb")
                oi_sb = opool.tile([P, NW], f16, name=f"oi{p}", tag="oi_sb")
                nc.scalar.copy(u[:], M1[:])
                nc.scalar.copy(v[:], M2[:])
                nc.vector.scalar_tensor_tensor(t[:], M3[:], 0.0, u[:], BYP, SUB)
                nc.vector.scalar_tensor_tensor(or_sb[:], t[:], 0.0, v[:], BYP, SUB)
                nc.sync.dma_start(or_d[ms, nsl(n)], or_sb[:])
                nc.gpsimd.tensor_sub(oi_sb[:], v[:], u[:])
                nc.sync.dma_start(oi_d[ms, nsl(n)], oi_sb[:])
                if n == 1 and m < MT // 2:
                    # negated out_i block for the host-side antisymmetric
                    # mirror (exact sign flip, gpsimd is SBUF-only)
                    oin_sb = opool.tile([P, NW], f16, name=f"oin{p}", tag="oin_sb")
                    nc.gpsimd.tensor_sub(oin_sb[:], zw[:, 0:NW], oi_sb[:])
                    nc.gpsimd.dma_start(oin_d[ms, :], oin_sb[:])

            # Passes 0/1 run k-interleaved with dummy fillers at the chunk
            # seams: late input chunks then can't open a gap in PE activity
            # (which would restart the HAM clock-gate wait). From pass 2 on,
            # inputs are resident and passes run straight through.
            alloc(0)
            alloc(1)
            mm_group(0, 0)
            mm_group(1, 0)
            filler(2 * P)
            mm_group(0, 1)
            mm_group(1, 1)
            filler(2 * P)
            filler(2 * P)
            mm_group(0, 2)
            mm_group(1, 2)
            filler(2 * P)
            filler(2 * P)
            mm_group(0, 3)
            evac(0)
            mm_group(1, 3)
            evac(1)
            for p in range(2, len(PASSES)):
                alloc(p)
                for k in range(KC):
                    mm_group(p, k)
                evac(p)

    nc.compile()
    return nc


def get_nc():
    if "nc" not in _CACHE:
        _CACHE["nc"] = _build()
    return _CACHE["nc"]


def make_in_maps(input_real, input_imag, weight):
    input_real = np.asarray(input_real)
    input_imag = np.asarray(input_imag)
    weight = np.asarray(weight, dtype=np.float32)
    # pack [S, D] -> [P, KC*D]: row p holds chunks k=0..KC-1 concatenated
    r16 = (
        input_real.astype(np.float16)
        .reshape(B, KC, P, D)
        .transpose(0, 2, 1, 3)
        .reshape(B, P, KC * D)
    )
    i16 = (
        input_imag.astype(np.float16)
        .reshape(B, KC, P, D)
        .transpose(0, 2, 1, 3)
        .reshape(B, P, KC * D)
    )
    # [B, P, KC]: col k = sqrt(w) for chunk k
    s_pack = np.sqrt(weight).astype(np.float32).reshape(B, KC, P).transpose(0, 2, 1)
    return [
        {
            "r_in": np.ascontiguousarray(r16[b]),
            "i_in": np.ascontiguousarray(i16[b]),
            "s_in": np.ascontiguousarray(s_pack[b]),
        }
        for b in range(B)
    ]


def unshard_single(or_np, oi_np, oin_np):
    """fp16 device outputs -> full fp32 [D,D] pair, mirroring the skipped
    strictly-lower blocks (pure transpose copies of device-computed data)."""
    out_r = np.asarray(or_np).astype(np.float32)
    out_i = np.asarray(oi_np).astype(np.float32)
    out_r[NW:D, 0:NW] = out_r[0:NW, NW:D].T
    out_i[NW:D, 0:NW] = np.asarray(oin_np).astype(np.float32).T
    return out_r, out_i


def run(input_real, input_imag, weight, **spmd_kwargs):
    nc = get_nc()
    res = bass_utils.run_bass_kernel_spmd(
        nc,
        make_in_maps(input_real, input_imag, weight),
        core_ids=list(range(N_CORES)),
        **spmd_kwargs,
    )
    outs = [
        unshard_single(
            res.results[b]["or_out"], res.results[b]["oi_out"],
            res.results[b]["oin_out"],
        )
        for b in range(B)
    ]
    out_r = np.stack([o[0] for o in outs])
    out_i = np.stack([o[1] for o in outs])
    return (out_r, out_i), res


def kernel(input_real, input_imag, weight):
    (out_r, out_i), _ = run(input_real, input_imag, weight)
    return (out_r, out_i)


# revision 9
# speedup vs baseline: 1.1704x; 1.1704x over previous
"""Trainium2 Bass kernel for nn_ComplexMixture.

Per batch element b (R = input_real[b] [S,D], I = input_imag[b] [S,D], w [S]):
    out_r = (w*R)^T R + (w*I)^T I        (symmetric)
    out_i = (w*I)^T R - (w*R)^T I        (antisymmetric)

Fold sqrt(w) into both operands (A = sqrt(w)*R, B = sqrt(w)*I) and use the
Gauss 3-multiplication complex product with E = A + B:
    M1 = A^T B,  M2 = B^T A,  M3 = E^T E
    out_r = M3 - M1 - M2
    out_i = M2 - M1
so each output block pair costs 3 PSUM-accumulated matmuls per contraction
chunk instead of 4 (25% less PE time). The combines run concurrently with
the matmul stream (only the vector engine may pair a PSUM read with a second
tensor operand; gpsimd may not touch PSUM at all):
    scalar: u = fp16(M1)                      (PSUM->SBUF copy)
    vector: t = M3 - u, out_r = t - M2, out_i = M2 - u

Sharding: data-parallel over batch, one batch element per NeuronCore (B == 8
== n_cores). Each core runs the identical program on its own slice.

Host marshalling: R/I are cast to fp16 (halves input DMA bytes) and sqrt(w)
is precomputed on host (4K scalars). Matmuls run in fp16 with fp32 PSUM
accumulation; outputs are stored as fp16 (halves store DMA bytes) and upcast
on host. Measured L2 relative error vs the fp32 reference ~5e-4.

out_r is symmetric and out_i antisymmetric, so each strictly-lower [384,384]
block is skipped on device (the pass list covers only the upper block
triangle). The host unshard mirrors them with pure transpose copies: out_r's
directly, out_i's from the device-negated oin_out.

Scheduling notes (Tile emits a static per-engine order from its own DMA
model; runtime queues are strictly in-order, and the PE HAM clock-gate
releases only after ~6us of continuous matmul activity — any gap restarts
the wait and leaves the PE at half rate):
  - inputs ride three rings in chunk order (r* on sync, i0 on scalar,
    s_t + i1-3 on gpsimd) so arrival matches consumption;
  - prep ops are spread over engines by deadline: early scales on vector,
    k2 scales on scalar, E0/E1 on gpsimd, E2/E3 back on vector;
  - the first two passes run k-major with the M3 matmuls trailing one
    chunk behind, plus dummy fillers at the chunk seams;
  - stores ride all three rings (or/sync, oi/scalar, oin/gpsimd).
"""

import sys
import types

import numpy as np

# If the environment requests tracing (BASS_TRACE=1) but the image lacks
# antenv.axon_hooks, bass_utils would crash importing it; provide a no-op
# hook registry so tracing degrades gracefully instead.
try:
    import antenv.axon_hooks  # noqa: F401
except ImportError:
    _hooks = types.ModuleType("antenv.axon_hooks")
    _hooks._hook = None
    _hooks.set_axon_ntff_profile_hook = lambda h: setattr(_hooks, "_hook", h)
    _hooks.get_axon_ntff_profile_hook = lambda: _hooks._hook
    sys.modules["antenv.axon_hooks"] = _hooks

import concourse.bacc as bacc
import concourse.bass_utils as bass_utils
import concourse.mybir as mybir
import concourse.tile as tile

B, S, D = 8, 512, 768
P = 128          # SBUF/PSUM partitions; matmul contraction tile
KC = S // P      # 4 contraction chunks per operand
MT = D // P      # 6 output row tiles
NW = 384         # matmul moving free dim (<=512 fp32 PSUM bank)
NB = D // NW     # 2 output column blocks
N_CORES = 8
N_PREWARM = 9    # dummy N=384 matmuls bridging the preamble barrier to the
                 # first real matmuls (~0.32us each at half rate)

# upper-block-triangle passes (m row tile, n 384-col block); strictly-lower
# blocks are mirrored on host from symmetry
PASSES = [(0, 0), (0, 1), (1, 0), (1, 1), (2, 0), (2, 1), (3, 1), (4, 1), (5, 1)]

_CACHE: dict = {}


def _build():
    f32, f16 = mybir.dt.float32, mybir.dt.float16
    BYP = mybir.AluOpType.bypass
    SUB = mybir.AluOpType.subtract
    nc = bacc.Bacc(
        "TRN2", target_bir_lowering=False, debug=False, num_devices=N_CORES
    )
    # Host-packed partition-major: r_in[p, k*D:(k+1)*D] = R[k*P+p, :], so a
    # whole k-chunk group is one DMA with long per-partition descriptors.
    r_d = nc.dram_tensor("r_in", [P, KC * D], f16, kind="ExternalInput").ap()
    i_d = nc.dram_tensor("i_in", [P, KC * D], f16, kind="ExternalInput").ap()
    # sqrt(w) chunks, partition-major (col k = chunk k's 128 scalars)
    s_d = nc.dram_tensor("s_in", [P, KC], f32, kind="ExternalInput").ap()
    or_d = nc.dram_tensor("or_out", [D, D], f16, kind="ExternalOutput").ap()
    oi_d = nc.dram_tensor("oi_out", [D, D], f16, kind="ExternalOutput").ap()
    # negated upper-right block of out_i; host transposes it into the
    # skipped lower-left block (out_i is antisymmetric)
    oin_d = nc.dram_tensor("oin_out", [D // 2, NW], f16, kind="ExternalOutput").ap()

    with tile.TileContext(nc) as tc:
        with (
            tc.tile_pool(name="const", bufs=1) as cpool,
            tc.tile_pool(name="stage", bufs=1) as spool,
            tc.tile_pool(name="abc", bufs=1) as apool,
            tc.tile_pool(name="tsb", bufs=2) as tpool,
            tc.tile_pool(name="osb", bufs=2) as opool,
            tc.tile_pool(name="ps2", bufs=2, space="PSUM") as ps2pool,
            tc.tile_pool(name="ps3", bufs=3, space="PSUM") as ps3pool,
            tc.tile_pool(name="pw", bufs=1, space="PSUM") as pwpool,
        ):
            # Scale vector first on the gpsimd ring: tiny (2KB), lands
            # before the i1-3 chunks queued behind it.
            s_t = cpool.tile([P, KC], f32, name="s_t")
            nc.gpsimd.dma_start(s_t[:], s_d)

            # PE prewarm (see module docstring).
            zw = cpool.tile([P, 4 * P], f16, name="zw")
            nc.vector.memset(zw[:], 0.0)
            pw_ps = pwpool.tile([P, 3 * P], f32, name="pw_ps", tag="pw")
            for _ in range(N_PREWARM):
                nc.tensor.matmul(
                    pw_ps[:], zw[:, 0:P], zw[:, P : 4 * P], start=True, stop=True
                )

            def filler():
                nc.tensor.matmul(
                    pw_ps[:, 0 : 2 * P], zw[:, 0:P], zw[:, P : 3 * P],
                    start=True, stop=True,
                )

            # One k-chunk per DMA, rings loaded in consumption order.
            rt, it = [], []
            for k in range(KC):
                rt.append(spool.tile([P, D], f16, name=f"r{k}", tag=f"r{k}"))
                it.append(spool.tile([P, D], f16, name=f"i{k}", tag=f"i{k}"))

            def dsl(k):
                return slice(k * D, (k + 1) * D)

            nc.sync.dma_start(rt[0][:], r_d[:, dsl(0)])
            nc.scalar.dma_start(it[0][:], i_d[:, dsl(0)])
            nc.sync.dma_start(rt[1][:], r_d[:, dsl(1)])
            nc.gpsimd.dma_start(it[1][:], i_d[:, dsl(1)])
            nc.sync.dma_start(rt[2][:], r_d[:, dsl(2)])
            nc.gpsimd.dma_start(it[2][:], i_d[:, dsl(2)])
            nc.sync.dma_start(rt[3][:], r_d[:, dsl(3)])
            nc.gpsimd.dma_start(it[3][:], i_d[:, dsl(3)])

            # Per-row scales A/B and sums E, spread by deadline.
            At = [apool.tile([P, D], f16, name=f"A{k}", tag=f"A{k}") for k in range(KC)]
            Bt = [apool.tile([P, D], f16, name=f"B{k}", tag=f"B{k}") for k in range(KC)]
            Et = [apool.tile([P, D], f16, name=f"E{k}", tag=f"E{k}") for k in range(KC)]

            def scl(k):
                return s_t[:, k : k + 1]

            nc.vector.tensor_scalar_mul(At[0][:], rt[0][:], scl(0))
            nc.vector.tensor_scalar_mul(Bt[0][:], it[0][:], scl(0))
            nc.vector.tensor_scalar_mul(At[1][:], rt[1][:], scl(1))
            nc.vector.tensor_scalar_mul(Bt[1][:], it[1][:], scl(1))
            nc.vector.tensor_scalar_mul(At[3][:], rt[3][:], scl(3))
            nc.vector.tensor_scalar_mul(Bt[3][:], it[3][:], scl(3))
            nc.scalar.mul(At[2][:], rt[2][:], scl(2))
            nc.scalar.mul(Bt[2][:], it[2][:], scl(2))
            nc.gpsimd.tensor_add(Et[0][:], At[0][:], Bt[0][:])
            nc.gpsimd.tensor_add(Et[1][:], At[1][:], Bt[1][:])
            nc.vector.tensor_add(Et[2][:], At[2][:], Bt[2][:])
            nc.vector.tensor_add(Et[3][:], At[3][:], Bt[3][:])

            def nsl(n):
                return slice(n * NW, (n + 1) * NW)

            ps_of = {}

            def alloc(p):
                M1 = ps2pool.tile([P, NW], f32, name=f"M1_{p}", tag="M1")
                M2 = ps3pool.tile([P, NW], f32, name=f"M2_{p}", tag="M2")
                M3 = ps2pool.tile([P, NW], f32, name=f"M3_{p}", tag="M3")
                ps_of[p] = (M1, M2, M3)

            def mm(p, which, k, st, sp):
                m, n = PASSES[p]
                ms = slice(m * P, (m + 1) * P)
                M1, M2, M3 = ps_of[p]
                dst, lt, rh = {
                    1: (M1, At[k], Bt[k]),
                    2: (M2, Bt[k], At[k]),
                    3: (M3, Et[k], Et[k]),
                }[which]
                nc.tensor.matmul(
                    dst[:], lt[:, ms], rh[:, nsl(n)], start=st, stop=sp
                )

            def evac(p):
                """Combine pass p's PSUM banks and store (fp16)."""
                m, n = PASSES[p]
                ms = slice(m * P, (m + 1) * P)
                M1, M2, M3 = ps_of[p]
                u = tpool.tile([P, NW], f16, name=f"u{p}", tag="u")
                t = tpool.tile([P, NW], f32, name=f"t{p}", tag="t")
                or_sb = opool.tile([P, NW], f16, name=f"or{p}", tag="or_sb")
                oi_sb = opool.tile([P, NW], f16, name=f"oi{p}", tag="oi_sb")
                nc.scalar.copy(u[:], M1[:])
                nc.vector.scalar_tensor_tensor(t[:], M3[:], 0.0, u[:], BYP, SUB)
                nc.vector.scalar_tensor_tensor(or_sb[:], t[:], 0.0, M2[:], BYP, SUB)
                nc.sync.dma_start(or_d[ms, nsl(n)], or_sb[:])
                nc.vector.scalar_tensor_tensor(oi_sb[:], M2[:], 0.0, u[:], BYP, SUB)
                nc.scalar.dma_start(oi_d[ms, nsl(n)], oi_sb[:])
                if n == 1 and m < MT // 2:
                    # negated out_i block for the host-side antisymmetric
                    # mirror (exact sign flip; gpsimd is SBUF-only)
                    oin_sb = opool.tile([P, NW], f16, name=f"oin{p}", tag="oin_sb")
                    nc.gpsimd.tensor_sub(oin_sb[:], zw[:, 0:NW], oi_sb[:])
                    nc.gpsimd.dma_start(oin_d[ms, :], oin_sb[:])

            # Head: passes 0/1 run k-major with M3 trailing one chunk so the
            # gpsimd E sums and late chunks can't open a PE activity gap;
            # fillers pad the riskiest seams.
            alloc(0)
            alloc(1)
            for p in (0, 1):
                mm(p, 1, 0, True, False)
            for p in (0, 1):
                mm(p, 2, 0, True, False)
            filler()
            for p in (0, 1):
                mm(p, 1, 1, False, False)
            for p in (0, 1):
                mm(p, 2, 1, False, False)
            for p in (0, 1):
                mm(p, 3, 0, True, False)
            filler()
            for p in (0, 1):
                mm(p, 1, 2, False, False)
            for p in (0, 1):
                mm(p, 2, 2, False, False)
            for p in (0, 1):
                mm(p, 3, 1, False, False)
            filler()
            for p in (0, 1):
                mm(p, 1, 3, False, True)
            for p in (0, 1):
                mm(p, 2, 3, False, True)
            for p in (0, 1):
                mm(p, 3, 2, False, False)
            for p in (0, 1):
                mm(p, 3, 3, False, True)
            evac(0)
            evac(1)
            # Steady state: straight passes; stop group ordered M1,M3,M2 so
            # the evac chain (u needs M1, t needs M3) starts early.
            for p in range(2, len(PASSES)):
                alloc(p)
                for k in range(KC - 1):
                    for which in (1, 2, 3):
                        mm(p, which, k, k == 0, False)
                mm(p, 1, KC - 1, False, True)
                mm(p, 3, KC - 1, False, True)
                mm(p, 2, KC - 1, False, True)
                evac(p)

    nc.compile()
    return nc


def get_nc():
    if "nc" not in _CACHE:
        _CACHE["nc"] = _build()
    return _CACHE["nc"]


def make_in_maps(input_real, input_imag, weight):
    input_real = np.asarray(input_real)
    input_imag = np.asarray(input_imag)
    weight = np.asarray(weight, dtype=np.float32)
    # pack [S, D] -> [P, KC*D]: row p holds chunks k=0..KC-1 concatenated
    r16 = (
        input_real.astype(np.float16)
        .reshape(B, KC, P, D)
        .transpose(0, 2, 1, 3)
        .reshape(B, P, KC * D)
    )
    i16 = (
        input_imag.astype(np.float16)
        .reshape(B, KC, P, D)
        .transpose(0, 2, 1, 3)
        .reshape(B, P, KC * D)
    )
    # [B, P, KC]: col k = sqrt(w) for chunk k
    s_pack = np.sqrt(weight).astype(np.float32).reshape(B, KC, P).transpose(0, 2, 1)
    return [
        {
            "r_in": np.ascontiguousarray(r16[b]),
            "i_in": np.ascontiguousarray(i16[b]),
            "s_in": np.ascontiguousarray(s_pack[b]),
        }
        for b in range(B)
    ]


def unshard_single(or_np, oi_np, oin_np):
    """fp16 device outputs -> full fp32 [D,D] pair, mirroring the skipped
    strictly-lower blocks (pure transpose copies of device-computed data)."""
    out_r = np.asarray(or_np).astype(np.float32)
    out_i = np.asarray(oi_np).astype(np.float32)
    out_r[NW:D, 0:NW] = out_r[0:NW, NW:D].T
    out_i[NW:D, 0:NW] = np.asarray(oin_np).astype(np.float32).T
    return out_r, out_i


def run(input_real, input_imag, weight, **spmd_kwargs):
    nc = get_nc()
    res = bass_utils.run_bass_kernel_spmd(
        nc,
        make_in_maps(input_real, input_imag, weight),
        core_ids=list(range(N_CORES)),
        **spmd_kwargs,
    )
    outs = [
        unshard_single(
            res.results[b]["or_out"], res.results[b]["oi_out"],
            res.results[b]["oin_out"],
        )
        for b in range(B)
    ]
    out_r = np.stack([o[0] for o in outs])
    out_i = np.stack([o[1] for o in outs])
    return (out_r, out_i), res


def kernel(input_real, input_imag, weight):
    (out_r, out_i), _ = run(input_real, input_imag, weight)
    return (out_r, out_i)
